# revision 19
# baseline (speedup 1.0000x reference)
"""Causal self-attention with RoPE + attention sinks on 8 Trainium2 NeuronCores.

Sharding: core d handles batch b = d//4 and heads 3*(d%4) .. 3*(d%4)+2
(data parallel on B x tensor parallel on the 12 heads). Each core computes a
partial [T, C] projection output (c_proj contraction over its 3 heads); the
host sums the 4 head-group partials per batch and adds b_proj*rezero.

v4 (on top of the v2 bf16 pipeline):
 - phase A runs nch-outer/mc-inner and streams the xt remainder in 512-col
   nch-major slices: the first three QKV groups need only the head 512
   columns of xt, so the PE starts ~4us in and each later chunk's pieces
   land just before its groups issue. rot1[:, 0:512] (gating head 0's first
   scores) is also ready early.
 - RoPE tables are 64-row periodic: half is uploaded and the other 64
   partitions are filled by an SBUF-local DMA (halves that HBM traffic).
 - the per-head sink exp(s_h) joins the softmax denominator via the DVE add
   that evacuates the denominator row, not a PE rank-1 matmul.
 - measured dead ends kept out: fp8 DoubleRow only pays at 256-wide
   contraction (scores/PV top out at 128, and fp8 attention weights cost
   ~1.8e-2 rel err vs the 2e-2 gate); denser phase-B restructurings lost
   more to the PE power throttle (util limit drops ~0.81 -> ~0.67) than
   they saved in instruction count.
"""

import numpy as np
import ml_dtypes

import concourse.bass as bass
import concourse.mybir as mybir
import concourse.tile as tile
from concourse.vector_clock import ScopedClock
from concourse.bass_utils import run_bass_kernel_spmd

F32 = mybir.dt.float32
BF = mybir.dt.bfloat16
ALU = mybir.AluOpType
ACTF = mybir.ActivationFunctionType
BF_NP = ml_dtypes.bfloat16

N_HEAD = 12
N_EMBD = 768
T = 2048
B = 2
HD = 64
HL = 3  # heads per core
NCORES = 8
THETA = 10000.0

KT = 6  # contraction tiles over C=768
NCH = 4  # 512-wide column chunks over T
TTILES = 16  # 128-row tiles over T

LAST_RESULTS = None  # BassKernelResults of the most recent run (for test.py)


class _TC(tile.TileContext):
    """TileContext whose tail drain splits its sem waits across multiple
    drain instructions; this walrus build rejects >1 wait on an SP Drain."""

    def _drain_and_barrier(self, tick_clock, wait_clock):
        drain_inst = self.nc.sync.drain()
        wait_clock.add_sem_waits(
            drain_inst.ins, ScopedClock({None: tick_clock.global_clock})
        )
        si = drain_inst.ins.sync_info
        if si is not None and len(si.on_wait) > 1:
            waits = list(si.on_wait)
            si.on_wait = waits[:1]
            for w in waits[1:]:
                d2 = self.nc.sync.drain()
                d2.ins.sync_info = mybir.SyncInfo(on_wait=[w], on_update=[])
        self.nc.all_engine_barrier()
        assert self.sems is not None
        popped = self.nc._tile_sem_poison_stack.pop()
        assert popped is self._sem_poison
        self.nc.clear_and_free_semaphores(list(self.sems.allocated().values()))
        self.nc.all_engine_barrier()


def _split_waits(nc, max_waits=1):
    """This walrus build accepts at most one sem-wait per TPB instruction.
    Hoist excess waits of any instruction onto NoOps inserted just before it
    on the same engine (in-order execution keeps semantics identical)."""
    for blk in nc.m.functions[0].blocks:
        new_insts = []
        for inst in blk.instructions:
            si = inst.sync_info
            if si is not None and len(si.on_wait) > max_waits:
                waits = list(si.on_wait)
                extra, keep = waits[:-max_waits], waits[-max_waits:]
                for i in range(0, len(extra), max_waits):
                    nop = mybir.InstNoOp(
                        name=nc.get_next_instruction_name(),
                        engine=inst.engine,
                        ins=[],
                        outs=[],
                        sync_info=mybir.SyncInfo(
                            on_wait=extra[i : i + max_waits], on_update=[]
                        ),
                    )
                    nc.register_instruction(nop)
                    new_insts.append(nop)
                si.on_wait = keep
            new_insts.append(inst)
        blk.instructions[:] = new_insts


def build_nc():
    nc = bass.Bass()

    xt_d = nc.dram_tensor("xt", [N_EMBD, T], BF, kind="ExternalInput")
    wqk_d = nc.dram_tensor("wqk", [N_EMBD, 384], BF, kind="ExternalInput")
    bqk_d = nc.dram_tensor("bqk", [128, 3], BF, kind="ExternalInput")
    bqksw_d = nc.dram_tensor("bqksw", [128, 3], BF, kind="ExternalInput")
    wv_d = nc.dram_tensor("wv", [N_EMBD, 256], BF, kind="ExternalInput")
    bv_d = nc.dram_tensor("bv", [1, 256], BF, kind="ExternalInput")
    wp_d = nc.dram_tensor("wp", [128, 3, N_EMBD], BF, kind="ExternalInput")
    ct_d = nc.dram_tensor("ct", [64, T], BF, kind="ExternalInput")
    st_d = nc.dram_tensor("st", [64, T], BF, kind="ExternalInput")
    perm_d = nc.dram_tensor("perm", [128, 128], BF, kind="ExternalInput")
    mir_d = nc.dram_tensor("mir", [128, 64], BF, kind="ExternalInput")
    sinkbc_d = nc.dram_tensor("sinkbc", [1, 1536], F32, kind="ExternalInput")
    ones_d = nc.dram_tensor("ones", [1, 512], BF, kind="ExternalInput")
    zeros_d = nc.dram_tensor("zeros1", [1, T], BF, kind="ExternalInput")
    rscr_d = nc.dram_tensor("rscr", [12, 512], F32, kind="Internal")
    yp_d = nc.dram_tensor("yp", [T, N_EMBD], BF, kind="ExternalOutput")

    with _TC(nc) as tc:
        with (
            tc.tile_pool(name="consts", bufs=1) as consts,
            tc.tile_pool(name="rot", bufs=1) as rotp,
            tc.tile_pool(name="vsb", bufs=1) as vp,
            tc.tile_pool(name="ytp", bufs=1) as ytp,
            tc.tile_pool(name="abuf", bufs=1) as abuf,
        ):
            # ---- persistent tiles ----
            ones_sb = consts.tile([1, 512], BF, tag="ones")
            sinkbc_sb = consts.tile([1, 1536], F32, tag="sinkbc")
            wp_sb = consts.tile([128, 3, N_EMBD], BF, tag="wp")

            # rot1: rotated [k_h0|k_h1]; rot2: rotated [q_h2|k_h2];
            # rot3f: [k_h2 relocated to 0..63 | zeros].
            # zq0/1/2: per-head rotated q zero-padded to 128 partitions so the
            # scores matmuls contract over K=128.
            rot1 = rotp.tile([128, T], BF, tag="rot1")
            rot2 = rotp.tile([128, T], BF, tag="rot2")
            rot3f = rotp.tile([128, T], BF, tag="rot3f")
            zq = [rotp.tile([128, T], BF, tag=f"zq{i}", name=f"zq{i}") for i in range(3)]
            zsrc = bass.AP(tensor=zeros_d, offset=0, ap=[[0, 64], [1, T]])
            # v tiles [t-tile 128, 3 heads x (64 dims + ones col) padded to 256]
            v_sb = vp.tile([128, TTILES, 256], BF, tag="v")
            # per-head normalized yT, zero-padded to 128 partitions
            yt = [ytp.tile([128, T], BF, tag=f"yt{i}", name=f"yt{i}") for i in range(3)]

            # ================= Phase A: QKV + RoPE =================
            # The pair-swap matmul + RoPE multiply-adds for chunk N are
            # emitted after chunk N+1's QKV matmuls so the PE never stalls
            # waiting for the evacuation of chunk N.
            with (
                tc.tile_pool(name="phA", bufs=1) as phA,
                tc.tile_pool(name="psA", bufs=3, space="PSUM") as psA,
                tc.tile_pool(name="psSw", bufs=3, space="PSUM") as psSw,
                tc.tile_pool(name="evac", bufs=3) as evacp,
                tc.tile_pool(name="swev", bufs=3) as swevp,
                tc.tile_pool(name="tmps", bufs=3) as tmpp,
            ):
                # Loads in the order compute needs them: interleave the wqk
                # contraction slices with the first 512-col chunk of xt so the
                # first QKV accumulation group can finish ~4us in.
                wqk_sb = phA.tile([128, KT, 384], BF, tag="wqk")
                xt_sb = abuf.tile([128, KT, T], BF, tag="xt")
                wqk_r = wqk_d.rearrange("(ci p) m -> p ci m", p=128)
                nc.sync.dma_start(out=wqk_sb[:, :, 0:128], in_=wqk_r[:, :, 0:128])
                for ci in range(KT):
                    nc.sync.dma_start(
                        out=xt_sb[:, ci, 0:512],
                        in_=xt_d[128 * ci : 128 * (ci + 1), 0:512],
                    )
                nc.sync.dma_start(out=wqk_sb[:, :, 128:384], in_=wqk_r[:, :, 128:384])
                # small tensors rope needs early
                bqk_sb = phA.tile([128, 3], BF, tag="bqk")
                nc.scalar.dma_start(out=bqk_sb, in_=bqk_d[:, :])
                bqksw_sb = phA.tile([128, 3], BF, tag="bqksw")
                nc.scalar.dma_start(out=bqksw_sb, in_=bqksw_d[:, :])
                perm_sb = phA.tile([128, 128], BF, tag="perm")
                nc.scalar.dma_start(out=perm_sb, in_=perm_d[:, :])
                ct_sb = phA.tile([128, T], BF, tag="ct")
                st_sb = phA.tile([128, T], BF, tag="st")
                # rope tables are 64-row periodic: upload half, duplicate the
                # other 64 partitions with an SBUF-local DMA
                nc.gpsimd.dma_start(out=ct_sb[0:64, :], in_=ct_d[:, :])
                nc.gpsimd.dma_start(out=st_sb[0:64, :], in_=st_d[:, :])
                nc.gpsimd.dma_start(out=ct_sb[64:128, :], in_=ct_sb[0:64, :])
                nc.gpsimd.dma_start(out=st_sb[64:128, :], in_=st_sb[0:64, :])
                mir_sb = phA.tile([128, 64], BF, tag="mir")
                nc.gpsimd.dma_start(out=mir_sb, in_=mir_d[:, :])
                # rest of xt, nch-major 512-col slices: chunk nch's six
                # contraction pieces land just before its QKV groups need them
                for nch in range(1, NCH):
                    ns_ = slice(512 * nch, 512 * (nch + 1))
                    for ci in range(KT):
                        nc.sync.dma_start(
                            out=xt_sb[:, ci, ns_],
                            in_=xt_d[128 * ci : 128 * (ci + 1), ns_],
                        )
                # v weights, zero fills, sinks — needed from phase B on
                wv_sb = abuf.tile([128, KT, 256], BF, tag="wv")
                nc.gpsimd.dma_start(
                    out=wv_sb,
                    in_=wv_d.rearrange("(ci p) m -> p ci m", p=128),
                )
                bv_sb = abuf.tile([1, 256], BF, tag="bv")
                nc.gpsimd.dma_start(out=bv_sb, in_=bv_d[:, :])
                nc.gpsimd.dma_start(out=ones_sb, in_=ones_d[:, :])
                nc.gpsimd.dma_start(out=sinkbc_sb, in_=sinkbc_d[:, :])
                nc.gpsimd.dma_start(out=zq[0][64:128, :], in_=zsrc)
                nc.gpsimd.dma_start(out=zq[1][0:64, :], in_=zsrc)
                nc.gpsimd.dma_start(out=zq[2][64:128, :], in_=zsrc)
                nc.gpsimd.dma_start(out=rot3f[64:128, :], in_=zsrc)
                for i in range(3):
                    nc.sync.dma_start(out=yt[i][64:128, :], in_=zsrc)
                # wp is only read in phase C — last
                nc.sync.dma_start(out=wp_sb, in_=wp_d[:, :, :])

                def rope_tail(mc, nch, qk_sb):
                    ns = slice(512 * nch, 512 * (nch + 1))
                    sw_ps = psSw.tile([128, 512], F32, tag="sw", bufs=2, name=f"sw{mc}_{nch}")
                    nc.tensor.matmul(sw_ps, perm_sb, qk_sb, start=True, stop=True)
                    sw_sb = swevp.tile([128, 512], BF, tag="swev", name=f"swev{mc}_{nch}")
                    nc.scalar.copy(out=sw_sb, in_=sw_ps)
                    tmp1 = tmpp.tile([128, 512], BF, tag="t1", name=f"t1_{mc}_{nch}")
                    nc.vector.scalar_tensor_tensor(
                        out=tmp1,
                        in0=qk_sb,
                        scalar=bqk_sb[:, mc : mc + 1],
                        in1=ct_sb[:, ns],
                        op0=ALU.add,
                        op1=ALU.mult,
                    )
                    tmp2 = tmpp.tile([128, 512], BF, tag="t2", name=f"t2_{mc}_{nch}")
                    nc.vector.scalar_tensor_tensor(
                        out=tmp2,
                        in0=sw_sb,
                        scalar=bqksw_sb[:, mc : mc + 1],
                        in1=st_sb[:, ns],
                        op0=ALU.add,
                        op1=ALU.mult,
                    )
                    if mc == 0:
                        nc.vector.tensor_add(
                            out=zq[0][0:64, ns], in0=tmp1[0:64, :], in1=tmp2[0:64, :]
                        )
                        nc.vector.tensor_add(
                            out=zq[1][64:128, ns],
                            in0=tmp1[64:128, :],
                            in1=tmp2[64:128, :],
                        )
                    elif mc == 1:
                        nc.vector.tensor_add(out=rot1[:, ns], in0=tmp1, in1=tmp2)
                    else:
                        nc.vector.tensor_add(out=rot2[:, ns], in0=tmp1, in1=tmp2)
                        m_ps = psSw.tile(
                            [64, 512], F32, tag="mir", bufs=1, name=f"m{nch}"
                        )
                        nc.tensor.matmul(
                            m_ps, mir_sb, rot2[:, ns], start=True, stop=True
                        )
                        nc.vector.tensor_copy(out=rot3f[0:64, ns], in_=m_ps)
                        nc.vector.tensor_copy(out=zq[2][0:64, ns], in_=rot2[0:64, ns])

                pending = None
                # nch-outer: the three mc groups of chunk 0 need only the
                # first 512-col xt pieces, so the PE starts ~4us in and the
                # rope for rot1[:, 0:512] (gating head 0's first scores) is
                # ready early while the rest of xt streams.
                for nch in range(NCH):
                    for mc in range(3):
                        ns = slice(512 * nch, 512 * (nch + 1))
                        qk_ps = psA.tile(
                            [128, 512], F32, tag="qk", bufs=3, name=f"qk{mc}_{nch}"
                        )
                        for ci in range(KT):
                            nc.tensor.matmul(
                                qk_ps,
                                wqk_sb[:, ci, 128 * mc : 128 * (mc + 1)],
                                xt_sb[:, ci, ns],
                                start=(ci == 0),
                                stop=(ci == KT - 1),
                            )
                        qk_sb = evacp.tile(
                            [128, 512], BF, tag="qkev", name=f"qkev{mc}_{nch}"
                        )
                        nc.scalar.copy(out=qk_sb, in_=qk_ps)
                        if pending is not None:
                            rope_tail(*pending)
                        pending = (mc, nch, qk_sb)
                    if nch == 0:
                        # V tiles 0..3 need only the first xt chunks + wv:
                        # they fill the PE while the xt remainder streams in.
                        for ti in range(4):
                            va = psA.tile(
                                [128, 512], F32, tag="vps", bufs=2, name=f"vpsA{ti}"
                            )
                            v_ps = va[:, 0:256]
                            for ci in range(KT):
                                nc.tensor.matmul(
                                    v_ps,
                                    xt_sb[:, ci, 128 * ti : 128 * (ti + 1)],
                                    wv_sb[:, ci, :],
                                    start=(ci == 0),
                                    stop=False,
                                )
                            nc.tensor.matmul(
                                v_ps,
                                ones_sb[0:1, 0:128],
                                bv_sb,
                                start=False,
                                stop=True,
                            )
                            nc.scalar.copy(out=v_sb[:, ti, :], in_=v_ps)

                rope_tail(*pending)

            # ================= Phase B: attention =================
            # V-tile matmuls are interleaved with head 0's q-chunks: the PE
            # computes V while ACT runs the first exps. The normalization
            # tail of iteration N is emitted after iteration N+1's matmuls;
            # it runs entirely off ACT (DVE reciprocal + DMA broadcast +
            # DVE multiply).
            qsl = [zq[0], zq[1], zq[2]]
            ksl = [rot1, rot1, rot3f]
            with (
                tc.tile_pool(name="psS", bufs=2, space="PSUM") as psS,
                tc.tile_pool(name="psY", bufs=3, space="PSUM") as psY,
                tc.tile_pool(name="psV", bufs=1, space="PSUM") as psV,
                tc.tile_pool(name="pt", bufs=18) as ptp,
                tc.tile_pool(name="smax", bufs=3) as smaxp,
                tc.tile_pool(name="rbp", bufs=3) as rbp,
            ):

                def norm_tail(hp, qc, y_ps):
                    # Softmax normalization, SBUF-local: sink exp joins the
                    # denominator row on the DVE during evacuation, a small
                    # DMA reshapes to [128,4] for the full-lane DVE
                    # reciprocal, a second DMA gathers the bf16 reciprocal
                    # back to a row, and a PE rank-1 matmul broadcasts it
                    # across 64 partitions (sharing psV's bank). ~5us chain
                    # vs ~9us for the old DRAM bounce -- the last two chains
                    # are exposed at the kernel tail.
                    cs = slice(512 * qc, 512 * (qc + 1))
                    den_sb = smaxp.tile([1, 512], F32, tag="den", name=f"d{hp}_{qc}")
                    nc.vector.tensor_add(
                        out=den_sb,
                        in0=y_ps[64:65, :],
                        in1=sinkbc_sb[0:1, 512 * hp : 512 * (hp + 1)],
                    )
                    den_t = smaxp.tile([128, 4], F32, tag="dent", name=f"dt{hp}_{qc}")
                    nc.sync.dma_start(out=den_t, in_=den_sb[0:1, :])
                    r_t = smaxp.tile([128, 4], F32, tag="rt", name=f"rt{hp}_{qc}")
                    nc.vector.reciprocal(out=r_t, in_=den_t)
                    r_b = smaxp.tile([128, 4], BF, tag="rb4", name=f"rb4{hp}_{qc}")
                    nc.vector.tensor_copy(out=r_b, in_=r_t)
                    rrow = smaxp.tile([1, 512], BF, tag="rrow", name=f"rr{hp}_{qc}")
                    nc.sync.dma_start(out=rrow, in_=r_b)
                    bc = psV.tile([128, 512], F32, tag="vps", name=f"bc{hp}_{qc}")
                    nc.tensor.matmul(
                        bc[0:64, :], ones_sb[0:1, 0:64], rrow, start=True, stop=True
                    )
                    rb = rbp.tile([64, 512], F32, tag="rb", name=f"rb{hp}_{qc}")
                    nc.vector.tensor_copy(out=rb, in_=bc[0:64, :])
                    nc.vector.tensor_mul(
                        out=yt[hp][0:64, 512 * qc : 512 * (qc + 1)],
                        in0=y_ps[0:64, :],
                        in1=rb,
                    )

                pending = None
                for hp in range(3):
                    qt = qsl[hp]
                    kt_ = ksl[hp]
                    for qc in range(NCH):
                        if hp == 0 and qc > 0:
                            # V tiles needed by this q-chunk's PV (and kept
                            # for the later heads); qc 0's were computed in
                            # phase A while the xt remainder streamed
                            for ti in range(4 * qc, 4 * qc + 4):
                                vx = psV.tile(
                                    [128, 512], F32, tag="vps", name=f"vps{ti}"
                                )
                                v_ps = vx[:, 0:256]
                                for ci in range(KT):
                                    nc.tensor.matmul(
                                        v_ps,
                                        xt_sb[:, ci, 128 * ti : 128 * (ti + 1)],
                                        wv_sb[:, ci, :],
                                        start=(ci == 0),
                                        stop=False,
                                    )
                                nc.tensor.matmul(
                                    v_ps,
                                    ones_sb[0:1, 0:128],
                                    bv_sb,
                                    start=False,
                                    stop=True,
                                )
                                nc.scalar.copy(out=v_sb[:, ti, :], in_=v_ps)
                        nki = 4 * qc + 4
                        nfull = 4 * qc  # k-tiles with no causal masking
                        pts = []
                        # full-width k-tiles processed in pairs: two score
                        # matmuls into one 2-bank PSUM tile, one Exp over both
                        for kp in range(0, nfull, 2):
                            st2 = psS.tile(
                                [128, 1024],
                                F32,
                                tag="st2",
                                bufs=2,
                                name=f"st2_{hp}_{qc}_{kp}",
                            )
                            pt2 = ptp.tile(
                                [128, 1024],
                                BF,
                                tag="pt",
                                name=f"pt{kp}",
                                bufs=9,
                            )
                            for j in range(2):
                                ki = kp + j
                                nc.tensor.matmul(
                                    st2[:, 512 * j : 512 * (j + 1)],
                                    kt_[:, 128 * ki : 128 * (ki + 1)],
                                    qt[:, 512 * qc : 512 * (qc + 1)],
                                    start=True,
                                    stop=True,
                                )
                            nc.scalar.activation(out=pt2, in_=st2, func=ACTF.Exp)
                            pts.append((pt2[:, 0:512], 0))
                            pts.append((pt2[:, 512:1024], 0))
                        # masked k-tiles in pairs as well: two score matmuls
                        # into one PSUM tile, one Exp, two affine_selects
                        for kp in range(nfull, nki, 2):
                            lefts = [max(0, 128 * (kp + j) - 512 * qc) for j in range(2)]
                            widths = [512 - lf for lf in lefts]
                            tot = widths[0] + widths[1]
                            st_ps = psS.tile(
                                [128, 1024],
                                F32,
                                tag="st2",
                                bufs=2,
                                name=f"st{hp}_{qc}_{kp}",
                            )
                            ptm = ptp.tile(
                                [128, 1024], BF, tag="ptm", name=f"ptm{kp}", bufs=3
                            )
                            off = 0
                            sub = []
                            for j in range(2):
                                ki = kp + j
                                nc.tensor.matmul(
                                    st_ps[:, off : off + widths[j]],
                                    kt_[:, 128 * ki : 128 * (ki + 1)],
                                    qt[:, 512 * qc + lefts[j] : 512 * (qc + 1)],
                                    start=True,
                                    stop=True,
                                )
                                sub.append((off, widths[j], lefts[j]))
                                off += widths[j]
                            nc.scalar.activation(
                                out=ptm[:, :tot], in_=st_ps[:, :tot], func=ACTF.Exp
                            )
                            for j in range(2):
                                off_j, w_j, lf_j = sub[j]
                                nc.gpsimd.affine_select(
                                    out=ptm[:, off_j : off_j + 128],
                                    in_=ptm[:, off_j : off_j + 128],
                                    pattern=[[1, 128]],
                                    base=0,
                                    channel_multiplier=-1,
                                    compare_op=ALU.is_ge,
                                    fill=0.0,
                                )
                                pts.append((ptm[:, off_j : off_j + w_j], lf_j))
                        y_ps = psY.tile([65, 512], F32, tag="y", name=f"y{hp}_{qc}")
                        for ki in range(nki):
                            rhs_ap, left = pts[ki]
                            nc.tensor.matmul(
                                y_ps[:, left:],
                                v_sb[:, ki, 65 * hp : 65 * hp + 65],
                                rhs_ap,
                                start=(ki == 0),
                                stop=(ki == nki - 1),
                            )
                        if pending is not None:
                            norm_tail(*pending)
                        pending = (hp, qc, y_ps)
                norm_tail(*pending)

            # ================= Phase C: projection =================
            with (
                tc.tile_pool(name="psP", bufs=3, space="PSUM") as psP,
                tc.tile_pool(name="pout", bufs=4) as poutp,
            ):
                for ti in range(TTILES):
                    p_sb = poutp.tile([128, N_EMBD], BF, tag="psb", name=f"psb{ti}")
                    for nn in range(2):
                        p_ps = psP.tile([128, 384], F32, tag="p", name=f"p{ti}_{nn}")
                        for hp in range(3):
                            nc.tensor.matmul(
                                p_ps,
                                yt[hp][:, 128 * ti : 128 * (ti + 1)],
                                wp_sb[:, hp, 384 * nn : 384 * (nn + 1)],
                                start=(hp == 0),
                                stop=(hp == 2),
                            )
                        if nn == 0:
                            nc.scalar.copy(
                                out=p_sb[:, 0:384], in_=p_ps
                            )
                        else:
                            nc.vector.tensor_copy(
                                out=p_sb[:, 384:768], in_=p_ps
                            )
                    nc.sync.dma_start(
                        out=yp_d[128 * ti : 128 * (ti + 1), :],
                        in_=p_sb,
                    )

    _split_waits(nc)
    return nc


_NC_CACHE = {}


def _get_nc():
    if "nc" not in _NC_CACHE:
        _NC_CACHE["nc"] = build_nc()
    return _NC_CACHE["nc"]


def _prep_core_inputs(inputs):
    """Host-side sharding: fold norm/scale/rezero into weights, build the
    per-core input maps (all device tensors in bf16)."""
    x = np.asarray(inputs["x"], np.float32)
    ns_ = np.asarray(inputs["norm_scale"], np.float32)
    nb_ = np.asarray(inputs["norm_bias"], np.float32)
    Wa = np.asarray(inputs["W_attn"], np.float32)
    ba = np.asarray(inputs["b_attn"], np.float32)
    Wp = np.asarray(inputs["W_proj"], np.float32)
    sinks = np.asarray(inputs["sinks"], np.float32)
    rz = float(np.asarray(inputs["rezero"], np.float32).reshape(()))

    C = N_EMBD
    W = (ns_[:, None] * Wa).astype(np.float32)
    beff = (nb_.astype(np.float64) @ Wa.astype(np.float64) + ba).astype(np.float32)
    scale = 1.0 / np.sqrt(np.float32(HD))

    # RoPE tables, interleaved-row layout (64-row periodic)
    freqs = (1.0 / THETA ** (np.arange(0, HD, 2, dtype=np.float64) / HD))  # [32]
    tpos = np.arange(T, dtype=np.float64)
    ang = np.outer(tpos, freqs)  # [T, 32]
    cos_t = np.cos(ang).T  # [32, T]
    sin_t = np.sin(ang).T
    ct = np.empty((64, T), np.float64)
    st = np.empty((64, T), np.float64)
    ct[0::2] = cos_t
    ct[1::2] = cos_t
    st[0::2] = -sin_t
    st[1::2] = sin_t
    ct = ct.astype(BF_NP)
    st = st.astype(BF_NP)

    perm = np.zeros((128, 128), np.float32)
    for i in range(64):
        perm[2 * i, 2 * i + 1] = 1.0
        perm[2 * i + 1, 2 * i] = 1.0
    mir = np.zeros((128, 64), np.float32)
    for i in range(64):
        mir[64 + i, i] = 1.0
    ones = np.ones((1, 512), np.float32)
    zeros1 = np.zeros((1, T), np.float32)
    swap_idx = np.arange(128)
    swap_idx = swap_idx + 1 - 2 * (swap_idx % 2)  # pairwise swap

    in_maps = []
    for d in range(NCORES):
        b = d // 4
        g = d % 4
        heads = [3 * g + j for j in range(HL)]

        wqk = np.empty((C, 384), np.float32)
        bqk = np.empty((128, 3), np.float32)
        # c0 = [q_h0 | q_h1], c1 = [k_h0 | k_h1], c2 = [q_h2 | k_h2]
        h0, h1, h2 = heads
        wqk[:, 0:64] = W[:, 64 * h0 : 64 * h0 + 64] * scale
        wqk[:, 64:128] = W[:, 64 * h1 : 64 * h1 + 64] * scale
        wqk[:, 128:192] = W[:, C + 64 * h0 : C + 64 * h0 + 64]
        wqk[:, 192:256] = W[:, C + 64 * h1 : C + 64 * h1 + 64]
        wqk[:, 256:320] = W[:, 64 * h2 : 64 * h2 + 64] * scale
        wqk[:, 320:384] = W[:, C + 64 * h2 : C + 64 * h2 + 64]
        bqk[0:64, 0] = beff[64 * h0 : 64 * h0 + 64] * scale
        bqk[64:128, 0] = beff[64 * h1 : 64 * h1 + 64] * scale
        bqk[0:64, 1] = beff[C + 64 * h0 : C + 64 * h0 + 64]
        bqk[64:128, 1] = beff[C + 64 * h1 : C + 64 * h1 + 64]
        bqk[0:64, 2] = beff[64 * h2 : 64 * h2 + 64] * scale
        bqk[64:128, 2] = beff[C + 64 * h2 : C + 64 * h2 + 64]
        bqksw = bqk[swap_idx, :].copy()

        wv = np.zeros((C, 256), np.float32)
        bv = np.zeros((1, 256), np.float32)
        for j, h in enumerate(heads):
            wv[:, 65 * j : 65 * j + 64] = W[:, 2 * C + 64 * h : 2 * C + 64 * h + 64]
            bv[0, 65 * j : 65 * j + 64] = beff[2 * C + 64 * h : 2 * C + 64 * h + 64]
            bv[0, 65 * j + 64] = 1.0

        wp = np.zeros((128, 3, C), np.float32)
        for j, h in enumerate(heads):
            wp[0:64, j, :] = Wp[64 * h : 64 * h + 64, :] * rz

        sinkbc = np.empty((1, 1536), np.float32)
        for j, h in enumerate(heads):
            sinkbc[0, 512 * j : 512 * (j + 1)] = np.exp(np.float64(sinks[h]))

        in_maps.append(
            {
                "xt": np.ascontiguousarray(x[b].T).astype(BF_NP),
                "wqk": wqk.astype(BF_NP),
                "bqk": bqk.astype(BF_NP),
                "bqksw": bqksw.astype(BF_NP),
                "wv": wv.astype(BF_NP),
                "bv": bv.astype(BF_NP),
                "wp": wp.astype(BF_NP),
                "ct": ct,
                "st": st,
                "perm": perm.astype(BF_NP),
                "mir": mir.astype(BF_NP),
                "sinkbc": sinkbc,
                "ones": ones.astype(BF_NP),
                "zeros1": zeros1.astype(BF_NP),
            }
        )

    bias_out = (np.asarray(inputs["b_proj"], np.float32) * rz).astype(np.float32)
    return in_maps, bias_out


def kernel(**inputs):
    global LAST_RESULTS
    nc = _get_nc()
    in_maps, bias_out = _prep_core_inputs(inputs)
    res = None
    last_exc = None
    for attempt in range(3):
        try:
            res = run_bass_kernel_spmd(nc, in_maps, core_ids=list(range(NCORES)))
            break
        except Exception as e:  # transient NRT_EXEC_UNIT_UNRECOVERABLE etc.
            last_exc = e
            import time as _time

            _time.sleep(2.0)
    if res is None:
        raise last_exc
    LAST_RESULTS = res
    y = np.zeros((B, T, N_EMBD), np.float32)
    for d in range(NCORES):
        y[d // 4] += np.asarray(res.results[d]["yp"], np.float32)
    y += bias_out[None, None, :]
    return y



# revision 21
# speedup vs baseline: 1.0340x; 1.0340x over previous
"""Causal self-attention with RoPE + attention sinks on 8 Trainium2 NeuronCores.

Sharding: core d handles batch b = d//4 and heads 3*(d%4) .. 3*(d%4)+2
(data parallel on B x tensor parallel on the 12 heads). Each core computes a
partial [T, C] projection output (c_proj contraction over its 3 heads); the
host sums the 4 head-group partials per batch and adds b_proj*rezero.

v4.6 (on top of the v2 bf16 pipeline):
 - phase A runs nch-outer/mc-inner with the wqk mc0 slice loaded first and
   the xt remainder streamed in 512-col nch-major slices, so the PE starts
   ~3us in and each chunk's pieces land just before its QKV groups issue.
 - DMA issue is spread across three rings: SP carries the wqk/xt stream,
   the ACT HWDGE ring carries the small early tensors (bqk/bqksw/perm), and
   the gpsimd SWDGE ring carries everything needed later (rope tables, wv,
   fills) -- SP issue serialization was gating the xt remainder.
 - RoPE tables are 64-row periodic: half is uploaded, the other 64
   partitions are filled by an SBUF-local DMA.
 - V-tile evacuations run on ACT (the PE, not ACT, is the wall now), which
   removed the DVE-rope-backlog stall at the phase A/B seam.
 - softmax normalization is SBUF-local: the sink exp joins the denominator
   row via the DVE evacuation add (no PE rank-1 sink matmul), the
   reciprocal runs full-lane on a [128,4] reshape, and a PE rank-1 matmul
   broadcasts the bf16 reciprocal row back across the head dims -- no DRAM
   round trip, which halved the normalization chains exposed at the tail.
 - measured dead ends kept out: fp8 DoubleRow only pays at 256-wide
   contraction (scores/PV top out at 128; fp8 attention weights cost
   ~1.8e-2 rel err vs the 2e-2 gate); denser phase-B restructurings and
   early V-tile hoisting lost more to the PE power throttle / ACT
   contention than they saved.
"""

import numpy as np
import ml_dtypes

import concourse.bass as bass
import concourse.mybir as mybir
import concourse.tile as tile
from concourse.vector_clock import ScopedClock
from concourse.bass_utils import run_bass_kernel_spmd

F32 = mybir.dt.float32
BF = mybir.dt.bfloat16
ALU = mybir.AluOpType
ACTF = mybir.ActivationFunctionType
BF_NP = ml_dtypes.bfloat16

N_HEAD = 12
N_EMBD = 768
T = 2048
B = 2
HD = 64
HL = 3  # heads per core
NCORES = 8
THETA = 10000.0

KT = 6  # contraction tiles over C=768
NCH = 4  # 512-wide column chunks over T
TTILES = 16  # 128-row tiles over T

LAST_RESULTS = None  # BassKernelResults of the most recent run (for test.py)


class _TC(tile.TileContext):
    """TileContext whose tail drain splits its sem waits across multiple
    drain instructions; this walrus build rejects >1 wait on an SP Drain."""

    def _drain_and_barrier(self, tick_clock, wait_clock):
        drain_inst = self.nc.sync.drain()
        wait_clock.add_sem_waits(
            drain_inst.ins, ScopedClock({None: tick_clock.global_clock})
        )
        si = drain_inst.ins.sync_info
        if si is not None and len(si.on_wait) > 1:
            waits = list(si.on_wait)
            si.on_wait = waits[:1]
            for w in waits[1:]:
                d2 = self.nc.sync.drain()
                d2.ins.sync_info = mybir.SyncInfo(on_wait=[w], on_update=[])
        self.nc.all_engine_barrier()
        assert self.sems is not None
        popped = self.nc._tile_sem_poison_stack.pop()
        assert popped is self._sem_poison
        self.nc.clear_and_free_semaphores(list(self.sems.allocated().values()))
        self.nc.all_engine_barrier()


def _split_waits(nc, max_waits=1):
    """This walrus build accepts at most one sem-wait per TPB instruction.
    Hoist excess waits of any instruction onto NoOps inserted just before it
    on the same engine (in-order execution keeps semantics identical)."""
    for blk in nc.m.functions[0].blocks:
        new_insts = []
        for inst in blk.instructions:
            si = inst.sync_info
            if si is not None and len(si.on_wait) > max_waits:
                waits = list(si.on_wait)
                extra, keep = waits[:-max_waits], waits[-max_waits:]
                for i in range(0, len(extra), max_waits):
                    nop = mybir.InstNoOp(
                        name=nc.get_next_instruction_name(),
                        engine=inst.engine,
                        ins=[],
                        outs=[],
                        sync_info=mybir.SyncInfo(
                            on_wait=extra[i : i + max_waits], on_update=[]
                        ),
                    )
                    nc.register_instruction(nop)
                    new_insts.append(nop)
                si.on_wait = keep
            new_insts.append(inst)
        blk.instructions[:] = new_insts


def build_nc():
    nc = bass.Bass()

    xt_d = nc.dram_tensor("xt", [N_EMBD, T], BF, kind="ExternalInput")
    wqk_d = nc.dram_tensor("wqk", [N_EMBD, 384], BF, kind="ExternalInput")
    bqk_d = nc.dram_tensor("bqk", [128, 3], BF, kind="ExternalInput")
    bqksw_d = nc.dram_tensor("bqksw", [128, 3], BF, kind="ExternalInput")
    wv_d = nc.dram_tensor("wv", [N_EMBD, 256], BF, kind="ExternalInput")
    bv_d = nc.dram_tensor("bv", [1, 256], BF, kind="ExternalInput")
    wp_d = nc.dram_tensor("wp", [128, 3, N_EMBD], BF, kind="ExternalInput")
    ct_d = nc.dram_tensor("ct", [64, T], BF, kind="ExternalInput")
    st_d = nc.dram_tensor("st", [64, T], BF, kind="ExternalInput")
    perm_d = nc.dram_tensor("perm", [128, 128], BF, kind="ExternalInput")
    mir_d = nc.dram_tensor("mir", [128, 64], BF, kind="ExternalInput")
    sinkbc_d = nc.dram_tensor("sinkbc", [1, 1536], F32, kind="ExternalInput")
    ones_d = nc.dram_tensor("ones", [1, 512], BF, kind="ExternalInput")
    zeros_d = nc.dram_tensor("zeros1", [1, T], BF, kind="ExternalInput")
    rscr_d = nc.dram_tensor("rscr", [12, 512], F32, kind="Internal")
    yp_d = nc.dram_tensor("yp", [T, N_EMBD], BF, kind="ExternalOutput")

    with _TC(nc) as tc:
        with (
            tc.tile_pool(name="consts", bufs=1) as consts,
            tc.tile_pool(name="rot", bufs=1) as rotp,
            tc.tile_pool(name="vsb", bufs=1) as vp,
            tc.tile_pool(name="ytp", bufs=1) as ytp,
            tc.tile_pool(name="abuf", bufs=1) as abuf,
        ):
            # ---- persistent tiles ----
            ones_sb = consts.tile([1, 512], BF, tag="ones")
            sinkbc_sb = consts.tile([1, 1536], F32, tag="sinkbc")
            wp_sb = consts.tile([128, 3, N_EMBD], BF, tag="wp")

            # rot1: rotated [k_h0|k_h1]; rot2: rotated [q_h2|k_h2];
            # rot3f: [k_h2 relocated to 0..63 | zeros].
            # zq0/1/2: per-head rotated q zero-padded to 128 partitions so the
            # scores matmuls contract over K=128.
            rot1 = rotp.tile([128, T], BF, tag="rot1")
            rot2 = rotp.tile([128, T], BF, tag="rot2")
            rot3f = rotp.tile([128, T], BF, tag="rot3f")
            zq = [rotp.tile([128, T], BF, tag=f"zq{i}", name=f"zq{i}") for i in range(3)]
            zsrc = bass.AP(tensor=zeros_d, offset=0, ap=[[0, 64], [1, T]])
            # v tiles [t-tile 128, 3 heads x (64 dims + ones col) padded to 256]
            v_sb = vp.tile([128, TTILES, 256], BF, tag="v")
            # per-head normalized yT, zero-padded to 128 partitions
            yt = [ytp.tile([128, T], BF, tag=f"yt{i}", name=f"yt{i}") for i in range(3)]

            # ================= Phase A: QKV + RoPE =================
            # The pair-swap matmul + RoPE multiply-adds for chunk N are
            # emitted after chunk N+1's QKV matmuls so the PE never stalls
            # waiting for the evacuation of chunk N.
            with (
                tc.tile_pool(name="phA", bufs=1) as phA,
                tc.tile_pool(name="psA", bufs=3, space="PSUM") as psA,
                tc.tile_pool(name="psSw", bufs=3, space="PSUM") as psSw,
                tc.tile_pool(name="evac", bufs=3) as evacp,
                tc.tile_pool(name="swev", bufs=3) as swevp,
                tc.tile_pool(name="tmps", bufs=3) as tmpp,
            ):
                # Loads in the order compute needs them: interleave the wqk
                # contraction slices with the first 512-col chunk of xt so the
                # first QKV accumulation group can finish ~4us in.
                wqk_sb = phA.tile([128, KT, 384], BF, tag="wqk")
                xt_sb = abuf.tile([128, KT, T], BF, tag="xt")
                wqk_r = wqk_d.rearrange("(ci p) m -> p ci m", p=128)
                nc.sync.dma_start(out=wqk_sb[:, :, 0:128], in_=wqk_r[:, :, 0:128])
                for ci in range(KT):
                    nc.sync.dma_start(
                        out=xt_sb[:, ci, 0:512],
                        in_=xt_d[128 * ci : 128 * (ci + 1), 0:512],
                    )
                nc.sync.dma_start(out=wqk_sb[:, :, 128:384], in_=wqk_r[:, :, 128:384])
                # small tensors rope needs early
                bqk_sb = phA.tile([128, 3], BF, tag="bqk")
                nc.scalar.dma_start(out=bqk_sb, in_=bqk_d[:, :])
                bqksw_sb = phA.tile([128, 3], BF, tag="bqksw")
                nc.scalar.dma_start(out=bqksw_sb, in_=bqksw_d[:, :])
                perm_sb = phA.tile([128, 128], BF, tag="perm")
                nc.scalar.dma_start(out=perm_sb, in_=perm_d[:, :])
                ct_sb = phA.tile([128, T], BF, tag="ct")
                st_sb = phA.tile([128, T], BF, tag="st")
                # rope tables are 64-row periodic: upload half, duplicate the
                # other 64 partitions with an SBUF-local DMA
                nc.gpsimd.dma_start(out=ct_sb[0:64, :], in_=ct_d[:, :])
                nc.gpsimd.dma_start(out=st_sb[0:64, :], in_=st_d[:, :])
                nc.gpsimd.dma_start(out=ct_sb[64:128, :], in_=ct_sb[0:64, :])
                nc.gpsimd.dma_start(out=st_sb[64:128, :], in_=st_sb[0:64, :])
                mir_sb = phA.tile([128, 64], BF, tag="mir")
                nc.gpsimd.dma_start(out=mir_sb, in_=mir_d[:, :])
                # rest of xt, nch-major 512-col slices: chunk nch's six
                # contraction pieces land just before its QKV groups need them
                for nch in range(1, NCH):
                    ns_ = slice(512 * nch, 512 * (nch + 1))
                    for ci in range(KT):
                        nc.sync.dma_start(
                            out=xt_sb[:, ci, ns_],
                            in_=xt_d[128 * ci : 128 * (ci + 1), ns_],
                        )
                # v weights, zero fills, sinks — needed from phase B on
                wv_sb = abuf.tile([128, KT, 256], BF, tag="wv")
                nc.gpsimd.dma_start(
                    out=wv_sb,
                    in_=wv_d.rearrange("(ci p) m -> p ci m", p=128),
                )
                bv_sb = abuf.tile([1, 256], BF, tag="bv")
                nc.gpsimd.dma_start(out=bv_sb, in_=bv_d[:, :])
                nc.gpsimd.dma_start(out=ones_sb, in_=ones_d[:, :])
                nc.gpsimd.dma_start(out=sinkbc_sb, in_=sinkbc_d[:, :])
                nc.gpsimd.dma_start(out=zq[0][64:128, :], in_=zsrc)
                nc.gpsimd.dma_start(out=zq[1][0:64, :], in_=zsrc)
                nc.gpsimd.dma_start(out=zq[2][64:128, :], in_=zsrc)
                nc.gpsimd.dma_start(out=rot3f[64:128, :], in_=zsrc)
                for i in range(3):
                    nc.sync.dma_start(out=yt[i][64:128, :], in_=zsrc)
                # wp is only read in phase C — last
                nc.sync.dma_start(out=wp_sb, in_=wp_d[:, :, :])

                def rope_tail(mc, nch, qk_sb):
                    ns = slice(512 * nch, 512 * (nch + 1))
                    sw_ps = psSw.tile([128, 512], F32, tag="sw", bufs=2, name=f"sw{mc}_{nch}")
                    nc.tensor.matmul(sw_ps, perm_sb, qk_sb, start=True, stop=True)
                    sw_sb = swevp.tile([128, 512], BF, tag="swev", name=f"swev{mc}_{nch}")
                    nc.scalar.copy(out=sw_sb, in_=sw_ps)
                    tmp1 = tmpp.tile([128, 512], BF, tag="t1", name=f"t1_{mc}_{nch}")
                    nc.vector.scalar_tensor_tensor(
                        out=tmp1,
                        in0=qk_sb,
                        scalar=bqk_sb[:, mc : mc + 1],
                        in1=ct_sb[:, ns],
                        op0=ALU.add,
                        op1=ALU.mult,
                    )
                    tmp2 = tmpp.tile([128, 512], BF, tag="t2", name=f"t2_{mc}_{nch}")
                    nc.vector.scalar_tensor_tensor(
                        out=tmp2,
                        in0=sw_sb,
                        scalar=bqksw_sb[:, mc : mc + 1],
                        in1=st_sb[:, ns],
                        op0=ALU.add,
                        op1=ALU.mult,
                    )
                    if mc == 0:
                        nc.vector.tensor_add(
                            out=zq[0][0:64, ns], in0=tmp1[0:64, :], in1=tmp2[0:64, :]
                        )
                        nc.vector.tensor_add(
                            out=zq[1][64:128, ns],
                            in0=tmp1[64:128, :],
                            in1=tmp2[64:128, :],
                        )
                    elif mc == 1:
                        nc.vector.tensor_add(out=rot1[:, ns], in0=tmp1, in1=tmp2)
                    else:
                        nc.vector.tensor_add(out=rot2[:, ns], in0=tmp1, in1=tmp2)
                        m_ps = psSw.tile(
                            [64, 512], F32, tag="mir", bufs=1, name=f"m{nch}"
                        )
                        nc.tensor.matmul(
                            m_ps, mir_sb, rot2[:, ns], start=True, stop=True
                        )
                        nc.vector.tensor_copy(out=rot3f[0:64, ns], in_=m_ps)
                        nc.vector.tensor_copy(out=zq[2][0:64, ns], in_=rot2[0:64, ns])

                pending = None
                # nch-outer: the three mc groups of chunk 0 need only the
                # first 512-col xt pieces, so the PE starts ~4us in and the
                # rope for rot1[:, 0:512] (gating head 0's first scores) is
                # ready early while the rest of xt streams.
                for nch in range(NCH):
                    for mc in range(3):
                        ns = slice(512 * nch, 512 * (nch + 1))
                        qk_ps = psA.tile(
                            [128, 512], F32, tag="qk", bufs=3, name=f"qk{mc}_{nch}"
                        )
                        for ci in range(KT):
                            nc.tensor.matmul(
                                qk_ps,
                                wqk_sb[:, ci, 128 * mc : 128 * (mc + 1)],
                                xt_sb[:, ci, ns],
                                start=(ci == 0),
                                stop=(ci == KT - 1),
                            )
                        qk_sb = evacp.tile(
                            [128, 512], BF, tag="qkev", name=f"qkev{mc}_{nch}"
                        )
                        nc.scalar.copy(out=qk_sb, in_=qk_ps)
                        if pending is not None:
                            rope_tail(*pending)
                        pending = (mc, nch, qk_sb)

                rope_tail(*pending)

            # ================= Phase B: attention =================
            # V-tile matmuls are interleaved with head 0's q-chunks: the PE
            # computes V while ACT runs the first exps. The normalization
            # tail of iteration N is emitted after iteration N+1's matmuls;
            # it runs entirely off ACT (DVE reciprocal + DMA broadcast +
            # DVE multiply).
            qsl = [zq[0], zq[1], zq[2]]
            ksl = [rot1, rot1, rot3f]
            with (
                tc.tile_pool(name="psS", bufs=2, space="PSUM") as psS,
                tc.tile_pool(name="psY", bufs=3, space="PSUM") as psY,
                tc.tile_pool(name="psV", bufs=1, space="PSUM") as psV,
                tc.tile_pool(name="pt", bufs=18) as ptp,
                tc.tile_pool(name="smax", bufs=3) as smaxp,
                tc.tile_pool(name="rbp", bufs=3) as rbp,
            ):

                def norm_tail(hp, qc, y_ps):
                    # Softmax normalization, SBUF-local: sink exp joins the
                    # denominator row on the DVE during evacuation, a small
                    # DMA reshapes to [128,4] for the full-lane DVE
                    # reciprocal, a second DMA gathers the bf16 reciprocal
                    # back to a row, and a PE rank-1 matmul broadcasts it
                    # across 64 partitions (sharing psV's bank). ~5us chain
                    # vs ~9us for the old DRAM bounce -- the last two chains
                    # are exposed at the kernel tail.
                    cs = slice(512 * qc, 512 * (qc + 1))
                    den_sb = smaxp.tile([1, 512], F32, tag="den", name=f"d{hp}_{qc}")
                    nc.vector.tensor_add(
                        out=den_sb,
                        in0=y_ps[64:65, :],
                        in1=sinkbc_sb[0:1, 512 * hp : 512 * (hp + 1)],
                    )
                    den_t = smaxp.tile([128, 4], F32, tag="dent", name=f"dt{hp}_{qc}")
                    nc.sync.dma_start(out=den_t, in_=den_sb[0:1, :])
                    r_t = smaxp.tile([128, 4], F32, tag="rt", name=f"rt{hp}_{qc}")
                    nc.vector.reciprocal(out=r_t, in_=den_t)
                    r_b = smaxp.tile([128, 4], BF, tag="rb4", name=f"rb4{hp}_{qc}")
                    nc.vector.tensor_copy(out=r_b, in_=r_t)
                    rrow = smaxp.tile([1, 512], BF, tag="rrow", name=f"rr{hp}_{qc}")
                    nc.sync.dma_start(out=rrow, in_=r_b)
                    bc = psV.tile([128, 512], F32, tag="vps", name=f"bc{hp}_{qc}")
                    nc.tensor.matmul(
                        bc[0:64, :], ones_sb[0:1, 0:64], rrow, start=True, stop=True
                    )
                    rb = rbp.tile([64, 512], F32, tag="rb", name=f"rb{hp}_{qc}")
                    nc.vector.tensor_copy(out=rb, in_=bc[0:64, :])
                    nc.vector.tensor_mul(
                        out=yt[hp][0:64, 512 * qc : 512 * (qc + 1)],
                        in0=y_ps[0:64, :],
                        in1=rb,
                    )

                pending = None
                for hp in range(3):
                    qt = qsl[hp]
                    kt_ = ksl[hp]
                    for qc in range(NCH):
                        if hp == 0:
                            # V tiles needed by this q-chunk's PV (and kept
                            # for the later heads)
                            for ti in range(4 * qc, 4 * qc + 4):
                                vx = psV.tile(
                                    [128, 512], F32, tag="vps", name=f"vps{ti}"
                                )
                                v_ps = vx[:, 0:256]
                                for ci in range(KT):
                                    nc.tensor.matmul(
                                        v_ps,
                                        xt_sb[:, ci, 128 * ti : 128 * (ti + 1)],
                                        wv_sb[:, ci, :],
                                        start=(ci == 0),
                                        stop=False,
                                    )
                                nc.tensor.matmul(
                                    v_ps,
                                    ones_sb[0:1, 0:128],
                                    bv_sb,
                                    start=False,
                                    stop=True,
                                )
                                nc.scalar.copy(out=v_sb[:, ti, :], in_=v_ps)
                        nki = 4 * qc + 4
                        nfull = 4 * qc  # k-tiles with no causal masking
                        pts = []
                        # full-width k-tiles processed in pairs: two score
                        # matmuls into one 2-bank PSUM tile, one Exp over both
                        for kp in range(0, nfull, 2):
                            st2 = psS.tile(
                                [128, 1024],
                                F32,
                                tag="st2",
                                bufs=2,
                                name=f"st2_{hp}_{qc}_{kp}",
                            )
                            pt2 = ptp.tile(
                                [128, 1024],
                                BF,
                                tag="pt",
                                name=f"pt{kp}",
                                bufs=9,
                            )
                            for j in range(2):
                                ki = kp + j
                                nc.tensor.matmul(
                                    st2[:, 512 * j : 512 * (j + 1)],
                                    kt_[:, 128 * ki : 128 * (ki + 1)],
                                    qt[:, 512 * qc : 512 * (qc + 1)],
                                    start=True,
                                    stop=True,
                                )
                            nc.scalar.activation(out=pt2, in_=st2, func=ACTF.Exp)
                            pts.append((pt2[:, 0:512], 0))
                            pts.append((pt2[:, 512:1024], 0))
                        # masked k-tiles in pairs as well: two score matmuls
                        # into one PSUM tile, one Exp, two affine_selects
                        for kp in range(nfull, nki, 2):
                            lefts = [max(0, 128 * (kp + j) - 512 * qc) for j in range(2)]
                            widths = [512 - lf for lf in lefts]
                            tot = widths[0] + widths[1]
                            st_ps = psS.tile(
                                [128, 1024],
                                F32,
                                tag="st2",
                                bufs=2,
                                name=f"st{hp}_{qc}_{kp}",
                            )
                            ptm = ptp.tile(
                                [128, 1024], BF, tag="ptm", name=f"ptm{kp}", bufs=3
                            )
                            off = 0
                            sub = []
                            for j in range(2):
                                ki = kp + j
                                nc.tensor.matmul(
                                    st_ps[:, off : off + widths[j]],
                                    kt_[:, 128 * ki : 128 * (ki + 1)],
                                    qt[:, 512 * qc + lefts[j] : 512 * (qc + 1)],
                                    start=True,
                                    stop=True,
                                )
                                sub.append((off, widths[j], lefts[j]))
                                off += widths[j]
                            nc.scalar.activation(
                                out=ptm[:, :tot], in_=st_ps[:, :tot], func=ACTF.Exp
                            )
                            for j in range(2):
                                off_j, w_j, lf_j = sub[j]
                                nc.gpsimd.affine_select(
                                    out=ptm[:, off_j : off_j + 128],
                                    in_=ptm[:, off_j : off_j + 128],
                                    pattern=[[1, 128]],
                                    base=0,
                                    channel_multiplier=-1,
                                    compare_op=ALU.is_ge,
                                    fill=0.0,
                                )
                                pts.append((ptm[:, off_j : off_j + w_j], lf_j))
                        y_ps = psY.tile([65, 512], F32, tag="y", name=f"y{hp}_{qc}")
                        for ki in range(nki):
                            rhs_ap, left = pts[ki]
                            nc.tensor.matmul(
                                y_ps[:, left:],
                                v_sb[:, ki, 65 * hp : 65 * hp + 65],
                                rhs_ap,
                                start=(ki == 0),
                                stop=(ki == nki - 1),
                            )
                        if pending is not None:
                            norm_tail(*pending)
                        pending = (hp, qc, y_ps)
                norm_tail(*pending)

            # ================= Phase C: projection =================
            with (
                tc.tile_pool(name="psP", bufs=3, space="PSUM") as psP,
                tc.tile_pool(name="pout", bufs=4) as poutp,
            ):
                for ti in range(TTILES):
                    p_sb = poutp.tile([128, N_EMBD], BF, tag="psb", name=f"psb{ti}")
                    for nn in range(2):
                        p_ps = psP.tile([128, 384], F32, tag="p", name=f"p{ti}_{nn}")
                        for hp in range(3):
                            nc.tensor.matmul(
                                p_ps,
                                yt[hp][:, 128 * ti : 128 * (ti + 1)],
                                wp_sb[:, hp, 384 * nn : 384 * (nn + 1)],
                                start=(hp == 0),
                                stop=(hp == 2),
                            )
                        if nn == 0:
                            nc.scalar.copy(
                                out=p_sb[:, 0:384], in_=p_ps
                            )
                        else:
                            nc.vector.tensor_copy(
                                out=p_sb[:, 384:768], in_=p_ps
                            )
                    nc.sync.dma_start(
                        out=yp_d[128 * ti : 128 * (ti + 1), :],
                        in_=p_sb,
                    )

    _split_waits(nc)
    return nc


_NC_CACHE = {}


def _get_nc():
    if "nc" not in _NC_CACHE:
        _NC_CACHE["nc"] = build_nc()
    return _NC_CACHE["nc"]


def _prep_core_inputs(inputs):
    """Host-side sharding: fold norm/scale/rezero into weights, build the
    per-core input maps (all device tensors in bf16)."""
    x = np.asarray(inputs["x"], np.float32)
    ns_ = np.asarray(inputs["norm_scale"], np.float32)
    nb_ = np.asarray(inputs["norm_bias"], np.float32)
    Wa = np.asarray(inputs["W_attn"], np.float32)
    ba = np.asarray(inputs["b_attn"], np.float32)
    Wp = np.asarray(inputs["W_proj"], np.float32)
    sinks = np.asarray(inputs["sinks"], np.float32)
    rz = float(np.asarray(inputs["rezero"], np.float32).reshape(()))

    C = N_EMBD
    W = (ns_[:, None] * Wa).astype(np.float32)
    beff = (nb_.astype(np.float64) @ Wa.astype(np.float64) + ba).astype(np.float32)
    scale = 1.0 / np.sqrt(np.float32(HD))

    # RoPE tables, interleaved-row layout (64-row periodic)
    freqs = (1.0 / THETA ** (np.arange(0, HD, 2, dtype=np.float64) / HD))  # [32]
    tpos = np.arange(T, dtype=np.float64)
    ang = np.outer(tpos, freqs)  # [T, 32]
    cos_t = np.cos(ang).T  # [32, T]
    sin_t = np.sin(ang).T
    ct = np.empty((64, T), np.float64)
    st = np.empty((64, T), np.float64)
    ct[0::2] = cos_t
    ct[1::2] = cos_t
    st[0::2] = -sin_t
    st[1::2] = sin_t
    ct = ct.astype(BF_NP)
    st = st.astype(BF_NP)

    perm = np.zeros((128, 128), np.float32)
    for i in range(64):
        perm[2 * i, 2 * i + 1] = 1.0
        perm[2 * i + 1, 2 * i] = 1.0
    mir = np.zeros((128, 64), np.float32)
    for i in range(64):
        mir[64 + i, i] = 1.0
    ones = np.ones((1, 512), np.float32)
    zeros1 = np.zeros((1, T), np.float32)
    swap_idx = np.arange(128)
    swap_idx = swap_idx + 1 - 2 * (swap_idx % 2)  # pairwise swap

    in_maps = []
    for d in range(NCORES):
        b = d // 4
        g = d % 4
        heads = [3 * g + j for j in range(HL)]

        wqk = np.empty((C, 384), np.float32)
        bqk = np.empty((128, 3), np.float32)
        # c0 = [q_h0 | q_h1], c1 = [k_h0 | k_h1], c2 = [q_h2 | k_h2]
        h0, h1, h2 = heads
        wqk[:, 0:64] = W[:, 64 * h0 : 64 * h0 + 64] * scale
        wqk[:, 64:128] = W[:, 64 * h1 : 64 * h1 + 64] * scale
        wqk[:, 128:192] = W[:, C + 64 * h0 : C + 64 * h0 + 64]
        wqk[:, 192:256] = W[:, C + 64 * h1 : C + 64 * h1 + 64]
        wqk[:, 256:320] = W[:, 64 * h2 : 64 * h2 + 64] * scale
        wqk[:, 320:384] = W[:, C + 64 * h2 : C + 64 * h2 + 64]
        bqk[0:64, 0] = beff[64 * h0 : 64 * h0 + 64] * scale
        bqk[64:128, 0] = beff[64 * h1 : 64 * h1 + 64] * scale
        bqk[0:64, 1] = beff[C + 64 * h0 : C + 64 * h0 + 64]
        bqk[64:128, 1] = beff[C + 64 * h1 : C + 64 * h1 + 64]
        bqk[0:64, 2] = beff[64 * h2 : 64 * h2 + 64] * scale
        bqk[64:128, 2] = beff[C + 64 * h2 : C + 64 * h2 + 64]
        bqksw = bqk[swap_idx, :].copy()

        wv = np.zeros((C, 256), np.float32)
        bv = np.zeros((1, 256), np.float32)
        for j, h in enumerate(heads):
            wv[:, 65 * j : 65 * j + 64] = W[:, 2 * C + 64 * h : 2 * C + 64 * h + 64]
            bv[0, 65 * j : 65 * j + 64] = beff[2 * C + 64 * h : 2 * C + 64 * h + 64]
            bv[0, 65 * j + 64] = 1.0

        wp = np.zeros((128, 3, C), np.float32)
        for j, h in enumerate(heads):
            wp[0:64, j, :] = Wp[64 * h : 64 * h + 64, :] * rz

        sinkbc = np.empty((1, 1536), np.float32)
        for j, h in enumerate(heads):
            sinkbc[0, 512 * j : 512 * (j + 1)] = np.exp(np.float64(sinks[h]))

        in_maps.append(
            {
                "xt": np.ascontiguousarray(x[b].T).astype(BF_NP),
                "wqk": wqk.astype(BF_NP),
                "bqk": bqk.astype(BF_NP),
                "bqksw": bqksw.astype(BF_NP),
                "wv": wv.astype(BF_NP),
                "bv": bv.astype(BF_NP),
                "wp": wp.astype(BF_NP),
                "ct": ct,
                "st": st,
                "perm": perm.astype(BF_NP),
                "mir": mir.astype(BF_NP),
                "sinkbc": sinkbc,
                "ones": ones.astype(BF_NP),
                "zeros1": zeros1.astype(BF_NP),
            }
        )

    bias_out = (np.asarray(inputs["b_proj"], np.float32) * rz).astype(np.float32)
    return in_maps, bias_out


def kernel(**inputs):
    global LAST_RESULTS
    nc = _get_nc()
    in_maps, bias_out = _prep_core_inputs(inputs)
    res = None
    last_exc = None
    for attempt in range(3):
        try:
            res = run_bass_kernel_spmd(nc, in_maps, core_ids=list(range(NCORES)))
            break
        except Exception as e:  # transient NRT_EXEC_UNIT_UNRECOVERABLE etc.
            last_exc = e
            import time as _time

            _time.sleep(2.0)
    if res is None:
        raise last_exc
    LAST_RESULTS = res
    y = np.zeros((B, T, N_EMBD), np.float32)
    for d in range(NCORES):
        y[d // 4] += np.asarray(res.results[d]["yp"], np.float32)
    y += bias_out[None, None, :]
    return y

